# revision 5
# baseline (speedup 1.0000x reference)
"""Trainium2 Bass kernel for nn_Map_79748952752358 (dense_cnn), v2.

v3: r-sharded conv (as the original baseline) but with the replicated
feature tensor in fp8e4 (16.8MB instead of 33.5MB per core) and the lin1
weight slice in fp8e4 (8.4MB, fully prefetched).  No collectives: the
ncfw collective path measured 15-130us of nondeterministic wake/boot
latency, worse than simply halving the feature traffic with fp8.  LSTM
runs as 2 interleaved half-batches.

Key math folds (exact, done on host):
  - BN scale s=gamma/sqrt(var+eps) > 0 folded into lin1 weights/bias.
  - relu(x)+t maxed over r == max(max_r(x+t), max_r(t)); +t injected into
    the conv PSUM via a rank-1 matmul, the floor max_r(t) applied on host.
  - channel-max over 256 r is permutation invariant, so the AllToAll slot
    order (slot s = r-rows of source core s) needs no per-core fixup.
"""

import os
import numpy as np
import ml_dtypes

import concourse.bass as bass
import concourse.mybir as mybir
from concourse import tile
from concourse.tile import ScopedClock
from concourse.alu_op_type import AluOpType
from concourse.bass_utils import run_bass_kernel_spmd

BF16 = ml_dtypes.bfloat16

B, S, V, E, HID = 16, 40, 1004, 256, 256
C, R, HW2 = 1024, 256, 1024
BN_EPS = 1e-5
N_CORES = 8
RS = R // N_CORES    # 32 r-rows per core (lin1 shard)
BS = B // N_CORES    # 2 samples per core (conv shard)
P = 128
CH = RS * C * BS     # a2a block: 65536 bf16 elems = 128KB

AFT = mybir.ActivationFunctionType
AX = mybir.AxisListType


# ---------------------------------------------------------------------------
# Tile tail-drain patch: this walrus build accepts fewer sem waits per
# TPB_CTRL instruction than Tile's exit drain accumulates; split them into
# single-wait SP nops.
_drain_patched = False


def _patch_tile_drain():
    global _drain_patched
    if _drain_patched:
        return
    _drain_patched = True

    def _patched(self, tick_clock, wait_clock):
        nc = self.nc
        probe = nc.sync.nop(nofuse=True, hint="drain_wait_split")
        wait_clock.add_sem_waits(
            probe.ins, ScopedClock({None: tick_clock.global_clock})
        )
        si = probe.ins.sync_info
        waits = list(si.on_wait or []) if si is not None else []
        if len(waits) > 1:
            si.on_wait = waits[:1]
            for w in waits[1:]:
                n = nc.sync.nop(nofuse=True, hint="drain_wait_split")
                nsi = n.ins.sync_info
                if nsi is None:
                    import bass_rust

                    n.ins.sync_info = bass_rust.SyncInfo(on_wait=[w], on_update=[])
                else:
                    nsi.on_wait = [w]
        nc.sync.drain()
        nc.all_engine_barrier()
        assert self.sems is not None
        popped = nc._tile_sem_poison_stack.pop()
        assert popped is self._sem_poison
        nc.clear_and_free_semaphores(list(self.sems.allocated().values()))
        nc.all_engine_barrier()

    tile.TileContext._drain_and_barrier = _patched


_ws_counter = [0]


def _split_excess_waits(nc, limit=1):
    """Walrus on this image rejects instructions with more than ~2 sem waits.
    Move excess waits onto same-engine EventSemaphore carriers inserted just
    before the offending instruction."""
    import bass_rust

    for fn in nc.m.functions:
        for bb in fn.blocks:
            out = []
            for inst in bb.instructions:
                si = inst.sync_info
                waits = list(si.on_wait or []) if si is not None else []
                if len(waits) > limit:
                    for w in waits[:-limit]:
                        _ws_counter[0] += 1
                        carrier = mybir.InstEventSemaphore(
                            name=f"I-waitsplit-{_ws_counter[0]}",
                            opcode="EventSemaphore",
                            engine=inst.engine,
                            sync_info=bass_rust.SyncInfo(
                                on_wait=[w], on_update=[]),
                        )
                        out.append(carrier)
                    si.on_wait = waits[-limit:]
                out.append(inst)
            bb.instructions = out


# ---------------------------------------------------------------------------
def _build_program(slots):
    """Build the SPMD Bass program. `slots[b]` = length[b]-1 (compile-time)."""
    _patch_tile_drain()
    nc = bass.Bass("TRN2", target_bir_lowering=False, debug=False,
                   num_devices=N_CORES)
    dt = mybir.dt
    f32, bf16 = dt.float32, dt.bfloat16

    fp8 = dt.float8e4

    def din(name, shape, d=bf16):
        if d is None:
            d = fp8
        return nc.dram_tensor(name, shape, d, kind="ExternalInput").ap()

    # feature, all 16 samples, (b, c_in, c-tile, hw) fp8e4
    feat_d = din("feat", [B, P, 8, HW2], None)
    # packed bf16 constant block: embT | wihT | whhT | e2dT | eye |
    # b1 | delta | tpat | ones_row  (32-partition blocks zero-padded)
    PK = [2 * S * B, 16 * P, 16 * P, 4 * P, P, 8 * P, RS * B, P, HW2 // 2]
    pack_d = din("cpack", [P, sum(PK)])
    biasf_d = din("biasf", [P, 11], f32)  # biasg(8) | e2db(2) | tconv(1)
    w1_d = din("w1T", [P, 8 * RS * 2 * P], None)  # fp8e4 tiles (ct, r, kh)

    # out: [p=(j4,q32), (g4, n2, blk16)] f32 partial channel-max
    out_d = nc.dram_tensor("part_out", [RS, B * 2 * 16], f32,
                           kind="ExternalOutput").ap()


    with tile.TileContext(nc) as tc:
        with (
            tc.tile_pool(name="const", bufs=1) as cpool,
            tc.tile_pool(name="xg", bufs=1) as xgpool,
            tc.tile_pool(name="hist", bufs=1) as hpool,
            tc.tile_pool(name="gs", bufs=2) as gspool,
            tc.tile_pool(name="cell", bufs=1) as cellpool,
            tc.tile_pool(name="tmp", bufs=4) as tmppool,
            tc.tile_pool(name="w1c", bufs=7) as w1pool,
            tc.tile_pool(name="f1", bufs=1) as f1pool,
            tc.tile_pool(name="rcv", bufs=1) as rpool,
            tc.tile_pool(name="feat", bufs=11) as fpool,
            tc.tile_pool(name="vout", bufs=1) as vpool,
            tc.tile_pool(name="mx", bufs=4) as mxpool,
        ):
            # ---- PE warm-up first: junk matmuls on uninitialized SBUF ----
            # (content irrelevant; just drive the HAM busy-window so the PE
            # is at 2.4GHz when stage A starts)
            warm = cpool.tile([P, P], bf16, tag="warmmm")
            nc.vector.memset(warm[:], 0.25)
            with tc.tile_pool(name="wpsum", bufs=2, space="PSUM") as wpsum:
                for i in range(46):
                    wp = wpsum.tile([P, P], f32, tag="warm")
                    nc.tensor.matmul(wp[:], lhsT=warm[:], rhs=warm[:],
                                     start=True, stop=True)

            pack = cpool.tile([P, sum(PK)], bf16, tag="cpack")
            nc.sync.dma_start(out=pack[:], in_=pack_d)
            biasf = cpool.tile([P, 11], f32, tag="biasf")
            nc.sync.dma_start(out=biasf[:], in_=biasf_d)

            off = np.cumsum([0] + PK)
            embT = pack[:, off[0]:off[1]]
            wih = pack[:, off[1]:off[2]]
            whh = pack[:, off[2]:off[3]]
            e2dT = pack[:, off[3]:off[4]]
            eye = pack[:, off[4]:off[5]]
            b1 = pack[0:RS, off[5]:off[6]]
            delta = pack[0:RS, off[6]:off[7]]
            tpat = pack[0:RS, off[7]:off[8]]      # [32, 2*128] rank-1 t rows
            ones_row = pack[0:RS, off[8]:off[9]]
            biasg = biasf[:, 0:8]
            e2db = biasf[:, 8:10]
            tconv = biasf[:, 10:11]

            # ---- feature DMA (16.8MB fp8; streams during LSTM) -----------
            fbs = {}
            for b in range(B):
                fbs[b] = fpool.tile([P, 8 * HW2], mybir.dt.float8e4,
                                    tag="feat", name=f"fb{b}")
                nc.sync.dma_start(
                    out=fbs[b][:].rearrange("p (kc hw) -> p kc hw", kc=8),
                    in_=feat_d[b])

            # ---- Stage A: xg = w_ih @ x_t for all steps (+ gate bias) ----
            xg_s = xgpool.tile([P, 8 * S * B], bf16)
            NCH = 320
            with tc.tile_pool(name="xpsum", bufs=2, space="PSUM") as xpsum:
                for m in range(8):
                    for n in range(2):
                        ps = xpsum.tile([P, NCH], f32, tag="xg")
                        for ke in range(2):
                            nc.tensor.matmul(
                                ps[:],
                                lhsT=wih[:, (ke * 8 + m) * P:(ke * 8 + m + 1) * P],
                                rhs=embT[:, ke * S * B + n * NCH: ke * S * B + (n + 1) * NCH],
                                start=(ke == 0), stop=(ke == 1),
                            )
                        nc.scalar.activation(
                            out=xg_s[:, m * S * B + n * NCH: m * S * B + (n + 1) * NCH],
                            in_=ps[:], func=AFT.Identity, bias=biasg[:, m:m + 1],
                        )

            # ---- Stage B: LSTM recurrence, 2 interleaved half-batches ----
            # Group g = samples 8g..8g+7.  While group 0's elementwise chain
            # runs on ACT/DVE, the PE does group 1's matmuls, halving the
            # per-step serial-latency cost.
            # Per group: gates psum gp [128, (m8, b8)]; work tile W [128,
            # (tg 16 | c 16)] so one 32-col DVE mult computes t1|t2 at once.
            GB = B // 2  # 8 samples per group
            hist = hpool.tile([P, S * 2 * B], bf16)   # (t, kh, b) b global
            Wk = {g: cellpool.tile([P, 4 * GB], f32, tag=f"W{g}",
                                   name=f"Wk{g}")
                  for g in range(2)}
            xg_r = xg_s[:].rearrange("p (m t b) -> p m t b", m=8, t=S)
            hist_r = hist[:].rearrange("p (t kh b) -> p t kh b", t=S, kh=2)
            S_g = [max(slots[g * GB:(g + 1) * GB]) + 1 for g in range(2)]
            lstm_psum = tc.tile_pool(name="gpsum", bufs=4, space="PSUM")
            gpsum = lstm_psum.__enter__()
            for t in range(max(S_g)):
                for g in range(2):
                    if t >= S_g[g]:
                        continue
                    bsl = slice(g * GB, (g + 1) * GB)
                    gp = gpsum.tile([P, 8 * GB], f32, tag="gates", name=f"gp{g}")
                    nc.tensor.matmul(gp[:], lhsT=eye[:],
                                     rhs=xg_r[:, :, t, bsl],
                                     start=True, stop=(t == 0))
                    if t > 0:
                        for m in range(8):
                            for kh in range(2):
                                nc.tensor.matmul(
                                    gp[:, m * GB:(m + 1) * GB],
                                    lhsT=whh[:, (kh * 8 + m) * P:
                                             (kh * 8 + m + 1) * P],
                                    rhs=hist_r[:, t - 1, kh, bsl],
                                    start=False, stop=(m == 7 and kh == 1),
                                    skip_group_check=True,
                                )
                    # cols (m, b8): i=0:16, f=16:32, g=32:48, o=48:64
                    gs = gspool.tile([P, 8 * GB], f32, tag=f"gs{g}")
                    nc.scalar.activation(out=gs[:], in_=gp[:],
                                         func=AFT.Sigmoid)
                    W = Wk[g]  # cols: tg 0:16 | c 16:32 (16 = kh2*b8)
                    nc.vector.tensor_scalar(W[:, 0:2 * GB], gs[:, 4 * GB:6 * GB],
                                            2.0, -1.0,
                                            AluOpType.mult, AluOpType.add)
                    U = tmppool.tile([P, 4 * GB], f32, tag=f"U{g}")
                    nc.vector.tensor_tensor(U[:, 0:4 * GB], gs[:, 0:4 * GB],
                                            W[:], AluOpType.mult)
                    if t == 0:
                        nc.vector.tensor_copy(W[:, 2 * GB:4 * GB],
                                              U[:, 0:2 * GB])
                    else:
                        nc.vector.tensor_tensor(W[:, 2 * GB:4 * GB],
                                                U[:, 0:2 * GB],
                                                U[:, 2 * GB:4 * GB],
                                                AluOpType.add)
                    th = tmppool.tile([P, 2 * GB], bf16, tag=f"th{g}")
                    nc.scalar.activation(out=th[:], in_=W[:, 2 * GB:4 * GB],
                                         func=AFT.Tanh)
                    nc.gpsimd.tensor_tensor(
                        hist_r[:, t, :, bsl],
                        gs[:, 6 * GB:8 * GB].rearrange(
                            "p (kh b) -> p kh b", kh=2),
                        th[:].rearrange("p (kh b) -> p kh b", kh=2),
                        AluOpType.mult)


            # ---- capture final h per sample (compile-time slots) ---------
            h_fin = cellpool.tile([P, 2 * B], bf16, tag="hfin")  # (kh, b)
            hf_r = h_fin[:].rearrange("p (k b) -> p b k", k=2)
            for b in range(B):
                src = hist[:, slots[b] * 2 * B:(slots[b] + 1) * 2 * B]
                nc.gpsimd.tensor_copy(
                    hf_r[:, b], src.rearrange("p (k b) -> p b k", k=2)[:, b])

            # ---- e2d projection: instrT = tanh(e2d_w @ h + b) ------------
            instrT = cellpool.tile([P, 2 * B], bf16, tag="instrT")  # (kh, b)
            for m in range(2):
                pe2 = gpsum.tile([P, B], f32, tag="e2d")
                for kh in range(2):
                    nc.tensor.matmul(
                        pe2[:],
                        lhsT=e2dT[:, (kh * 2 + m) * P:(kh * 2 + m + 1) * P],
                        rhs=h_fin[:, kh * B:(kh + 1) * B],
                        start=(kh == 0), stop=(kh == 1),
                    )
                nc.scalar.activation(out=instrT[:, m * B:(m + 1) * B],
                                     in_=pe2[:], func=AFT.Tanh,
                                     bias=e2db[:, m:m + 1])
            lstm_psum.__exit__(None, None, None)

            # ---- lin1 (r-slice): f1[c, (ct,b,r)] = Lrelu(W @ instr + b1) -
            f1_sb = f1pool.tile([P, 8 * B * RS], mybir.dt.float8e4)
            CW = RS * 2 * P
            lin1_psum = tc.tile_pool(name="lpsum", bufs=4, space="PSUM")
            lpsum = lin1_psum.__enter__()
            for ct in range(8):
                wch = w1pool.tile([P, CW], mybir.dt.float8e4, tag="w1c")
                nc.sync.dma_start(out=wch[:], in_=w1_d[:, ct * CW:(ct + 1) * CW])
                pb = lpsum.tile([P, RS * B], f32, tag="lin1")
                nc.tensor.matmul(pb[:], lhsT=b1[:, ct * P:(ct + 1) * P],
                                 rhs=delta[:], start=True, stop=False,
                                 skip_group_check=True)
                for r in range(RS):
                    for kh in range(2):
                        nc.tensor.matmul(
                            pb[:, r * B:(r + 1) * B],
                            lhsT=wch[:, (r * 2 + kh) * P:(r * 2 + kh + 1) * P],
                            rhs=instrT[:, kh * B:(kh + 1) * B],
                            start=False, stop=(r == RS - 1 and kh == 1),
                            skip_group_check=True,
                        )
                out_ap = (f1_sb[:, ct * B * RS:(ct + 1) * B * RS]
                          .rearrange("p (b r) -> p r b", b=B))
                nc.scalar.activation(out=out_ap, in_=pb[:], func=AFT.Lrelu,
                                     alpha=0.01)
            lin1_psum.__exit__(None, None, None)
            # f1_sb col layout: ct*512 + b*32 + r

            # ---- conv + fused BN-shift + channel max (r-sharded) ---------
            # out[32r, hw] per sample; 4 samples packed in the 4 PE
            # column-quadrants via tile_position.  rhs = fp8 feature.
            vout = vpool.tile([RS, B * 2 * 16], f32)  # [q32, (b, n, blk)]
            conv_psum = tc.tile_pool(name="cpsum", bufs=4, space="PSUM")
            cpsum = conv_psum.__enter__()
            NH = HW2 // 2  # 512
            f1r = f1_sb[:].rearrange("p (kc b r) -> p kc b r", kc=8, b=B)
            for b in range(B):
                fb_r = fbs[b][:].rearrange("p (kc hw) -> p kc hw", kc=8)
                for n in range(2):
                    pc = cpsum.tile([RS, NH], f32, tag="conv")
                    for kp in range(4):
                        nc.tensor.matmul(
                            pc[:],
                            lhsT=f1r[:, 2 * kp:2 * kp + 2, b],
                            rhs=fb_r[:, 2 * kp:2 * kp + 2,
                                     n * NH:(n + 1) * NH],
                            start=(kp == 0), stop=(kp == 3),
                            skip_group_check=True,
                            perf_mode=mybir.MatmulPerfMode.DoubleRow,
                        )
                    cp = mxpool.tile([RS, NH], f32, tag="convcp")
                    nc.scalar.activation(out=cp[:], in_=pc[:],
                                         func=AFT.Identity,
                                         bias=tconv[0:RS])
                    nc.vector.tensor_reduce(
                        out=vout[0:RS, (b * 2 + n) * 16:(b * 2 + n + 1) * 16],
                        in_=cp[:].rearrange("p (blk q) -> p blk q", q=32),
                        axis=AX.X, op=AluOpType.max, apply_transpose=True)
            conv_psum.__exit__(None, None, None)

            nc.sync.dma_start(out=out_d, in_=vout[:])

    _split_excess_waits(nc)
    return nc


# ---------------------------------------------------------------------------
def _prep_inputs(feature, instruction_idx, instruction_length, emb_table,
                 w_ih, w_hh, b_ih, b_hh, e2d_w, e2d_b,
                 lin1_w, lin1_b, bn_gamma, bn_beta, bn_mean, bn_var):
    """Host-side layout/dtype prep. Returns (in_maps, slots, T0)."""
    f32 = np.float32

    def to_bf(x):
        return np.ascontiguousarray(x.astype(BF16))

    feature = np.asarray(feature, f32)
    emb_table = np.asarray(emb_table, f32)
    idx = np.asarray(instruction_idx)
    lengths = np.asarray(instruction_length).astype(np.int64)
    slots = [int(max(l, 1) - 1) for l in lengths]

    # feature [b, c_in(p), kc, hw] in fp8e4, replicated to all cores
    featr = feature.reshape(B, 8, P, HW2).transpose(0, 2, 1, 3)  # [B,P,8,HW2]
    feat8 = np.ascontiguousarray(featr.astype(ml_dtypes.float8_e4m3))

    # embeds transposed: [p, (ke, t*b)]
    emb = emb_table[idx]                       # [B, S, E]
    embT = emb.transpose(2, 1, 0).reshape(2, P, S * B)
    embT = to_bf(embT.transpose(1, 0, 2).reshape(P, 2 * S * B))

    def wtiles(w, kt, mt):
        wt = np.asarray(w, f32).T
        a = wt.reshape(kt, P, mt, P).transpose(1, 0, 2, 3)
        return to_bf(a.reshape(P, kt * mt * P))

    gsc = np.ones((4 * HID, 1), f32)
    gsc[2 * HID:3 * HID] = 2.0
    wihT = wtiles(np.asarray(w_ih, f32) * gsc, 2, 8)
    whhT = wtiles(np.asarray(w_hh, f32) * gsc, 2, 8)
    e2dT = wtiles(e2d_w, 2, 2)

    bg = ((np.asarray(b_ih, f32) + np.asarray(b_hh, f32)) * gsc[:, 0]) \
        .reshape(8, P).T.copy()
    e2db = np.asarray(e2d_b, f32).reshape(2, P).T.copy()

    s = np.asarray(bn_gamma, f32) / np.sqrt(np.asarray(bn_var, f32) + BN_EPS)
    tsh = np.asarray(bn_beta, f32) - np.asarray(bn_mean, f32) * s
    T0 = float(tsh.max())

    w1s = np.asarray(lin1_w, f32).reshape(R, C, HID) * s[:, None, None]
    b1s = np.asarray(lin1_b, f32).reshape(R, C) * s[:, None]

    delta = np.repeat(np.eye(RS, dtype=f32), B, axis=1)  # [32, 512]
    eye = np.eye(P, dtype=f32)
    ones32 = np.ones((RS, P), f32)

    def pad128(a):
        out = np.zeros((P, a.shape[1]), f32)
        out[:a.shape[0]] = a
        return out

    biasf = np.concatenate([bg, e2db], axis=1).astype(f32)

    ones_row = np.zeros((RS, HW2 // 2), f32)
    ones_row[0] = 1.0

    in_maps = []
    for k in range(N_CORES):
        rsl = slice(k * RS, (k + 1) * RS)
        wsl = w1s[rsl]                          # [32, 1024, 256] (r, c, h)
        ws = wsl.transpose(2, 1, 0)             # [h, c, r]
        a = (ws.reshape(2, P, 8, P, RS)         # [kh, p, ct, col, r]
             .transpose(1, 2, 4, 0, 3)          # [p, ct, r, kh, col]
             .reshape(P, 8 * RS * 2 * P))
        b1c = b1s[rsl].reshape(RS, 8, P).reshape(RS, 8 * P)  # (r, (ct, c))
        # conv BN-shift injection: out[p,:] += t[p%32] via rank-1 matmul
        tpat = np.zeros((RS, P), f32)
        tpat[0] = np.tile(tsh[rsl], 4)
        cpack = np.concatenate(
            [embT.astype(f32), wihT.astype(f32), whhT.astype(f32),
             e2dT.astype(f32), eye, pad128(b1c), pad128(delta),
             pad128(tpat), pad128(ones_row)], axis=1)
        tcol = np.zeros((P, 1), f32)
        tcol[:RS, 0] = tsh[rsl]
        biasf_k = np.ascontiguousarray(np.concatenate([biasf, tcol], axis=1))
        in_maps.append(dict(
            feat=feat8, cpack=to_bf(cpack), biasf=biasf_k,
            w1T=np.ascontiguousarray(a.astype(ml_dtypes.float8_e4m3))))
    return in_maps, slots, T0


_cache = {}


def _run(inputs, trace=False):
    (in_maps, slots, T0) = _prep_inputs(
        inputs["feature"], inputs["instruction_idx"],
        inputs["instruction_length"], inputs["emb_table"],
        inputs["w_ih"], inputs["w_hh"], inputs["b_ih"], inputs["b_hh"],
        inputs["e2d_w"], inputs["e2d_b"], inputs["lin1_w"], inputs["lin1_b"],
        inputs["bn_gamma"], inputs["bn_beta"], inputs["bn_mean"],
        inputs["bn_var"])

    key = tuple(slots)
    if key not in _cache:
        _cache[key] = _build_program(slots)
    nc = _cache[key]

    kw = {}
    if trace:
        kw = dict(trace=True, trace_cores=list(range(N_CORES)))
    res = run_bass_kernel_spmd(nc, in_maps, list(range(N_CORES)), **kw)
    parts = np.stack([np.asarray(res.results[i]["part_out"], np.float32)
                      for i in range(N_CORES)])
    v = parts.reshape(N_CORES, 32, B, 2, 16)      # [core, q, b, n, blk]
    v = v.transpose(0, 2, 3, 4, 1)                # [core, b, n, blk, q]
    single = v.reshape(N_CORES, B, HW2).max(axis=0)
    single = np.maximum(single, T0)
    out = np.clip(single, 0.0, 1.0).reshape(B, 32, 32).astype(np.float32)
    return out, res


def kernel(**inputs) -> np.ndarray:
    out, _ = _run(inputs, trace=False)
    return out


def kernel_traced(**inputs):
    out, res = _run(inputs, trace=True)
    return out, res


# revision 6
# speedup vs baseline: 1.0190x; 1.0190x over previous
"""Trainium2 Bass kernel for nn_Map_79748952752358 (dense_cnn), v2.

v3: r-sharded conv (as the original baseline) but with the replicated
feature tensor in fp8e4 (16.8MB instead of 33.5MB per core) and the lin1
weight slice in fp8e4 (8.4MB, fully prefetched).  No collectives: the
ncfw collective path measured 15-130us of nondeterministic wake/boot
latency, worse than simply halving the feature traffic with fp8.  LSTM
runs as 2 interleaved half-batches.

Key math folds (exact, done on host):
  - BN scale s=gamma/sqrt(var+eps) > 0 folded into lin1 weights/bias.
  - relu(x)+t maxed over r == max(max_r(x+t), max_r(t)); +t injected into
    the conv PSUM via a rank-1 matmul, the floor max_r(t) applied on host.
  - channel-max over 256 r is permutation invariant, so the AllToAll slot
    order (slot s = r-rows of source core s) needs no per-core fixup.
"""

import os
import numpy as np
import ml_dtypes

import concourse.bass as bass
import concourse.mybir as mybir
from concourse import tile
from concourse.tile import ScopedClock
from concourse.alu_op_type import AluOpType
from concourse.bass_utils import run_bass_kernel_spmd

BF16 = ml_dtypes.bfloat16

B, S, V, E, HID = 16, 40, 1004, 256, 256
C, R, HW2 = 1024, 256, 1024
BN_EPS = 1e-5
N_CORES = 8
RS = R // N_CORES    # 32 r-rows per core (lin1 shard)
BS = B // N_CORES    # 2 samples per core (conv shard)
P = 128
CH = RS * C * BS     # a2a block: 65536 bf16 elems = 128KB

AFT = mybir.ActivationFunctionType
AX = mybir.AxisListType


# ---------------------------------------------------------------------------
# Tile tail-drain patch: this walrus build accepts fewer sem waits per
# TPB_CTRL instruction than Tile's exit drain accumulates; split them into
# single-wait SP nops.
_drain_patched = False


def _patch_tile_drain():
    global _drain_patched
    if _drain_patched:
        return
    _drain_patched = True

    def _patched(self, tick_clock, wait_clock):
        nc = self.nc
        probe = nc.sync.nop(nofuse=True, hint="drain_wait_split")
        wait_clock.add_sem_waits(
            probe.ins, ScopedClock({None: tick_clock.global_clock})
        )
        si = probe.ins.sync_info
        waits = list(si.on_wait or []) if si is not None else []
        if len(waits) > 1:
            si.on_wait = waits[:1]
            for w in waits[1:]:
                n = nc.sync.nop(nofuse=True, hint="drain_wait_split")
                nsi = n.ins.sync_info
                if nsi is None:
                    import bass_rust

                    n.ins.sync_info = bass_rust.SyncInfo(on_wait=[w], on_update=[])
                else:
                    nsi.on_wait = [w]
        nc.sync.drain()
        nc.all_engine_barrier()
        assert self.sems is not None
        popped = nc._tile_sem_poison_stack.pop()
        assert popped is self._sem_poison
        nc.clear_and_free_semaphores(list(self.sems.allocated().values()))
        nc.all_engine_barrier()

    tile.TileContext._drain_and_barrier = _patched


_ws_counter = [0]


def _split_excess_waits(nc, limit=1):
    """Walrus on this image rejects instructions with more than ~2 sem waits.
    Move excess waits onto same-engine EventSemaphore carriers inserted just
    before the offending instruction."""
    import bass_rust

    for fn in nc.m.functions:
        for bb in fn.blocks:
            out = []
            for inst in bb.instructions:
                si = inst.sync_info
                waits = list(si.on_wait or []) if si is not None else []
                if len(waits) > limit:
                    for w in waits[:-limit]:
                        _ws_counter[0] += 1
                        carrier = mybir.InstEventSemaphore(
                            name=f"I-waitsplit-{_ws_counter[0]}",
                            opcode="EventSemaphore",
                            engine=inst.engine,
                            sync_info=bass_rust.SyncInfo(
                                on_wait=[w], on_update=[]),
                        )
                        out.append(carrier)
                    si.on_wait = waits[-limit:]
                out.append(inst)
            bb.instructions = out


# ---------------------------------------------------------------------------
def _build_program(slots):
    """Build the SPMD Bass program. `slots[b]` = length[b]-1 (compile-time)."""
    _patch_tile_drain()
    nc = bass.Bass("TRN2", target_bir_lowering=False, debug=False,
                   num_devices=N_CORES)
    dt = mybir.dt
    f32, bf16 = dt.float32, dt.bfloat16

    fp8 = dt.float8e4

    def din(name, shape, d=bf16):
        if d is None:
            d = fp8
        return nc.dram_tensor(name, shape, d, kind="ExternalInput").ap()

    # feature, all 16 samples, (b, c_in, c-tile, hw) fp8e4
    feat_d = din("feat", [B, P, 8, HW2], None)
    # packed bf16 constant block: embT | wihT | whhT | e2dT | eye |
    # b1 | delta | tpat | ones_row  (32-partition blocks zero-padded)
    PK = [2 * S * B, 16 * P, 16 * P, 4 * P, P, 8 * P, RS * B, P, HW2 // 2]
    pack_d = din("cpack", [P, sum(PK)])
    biasf_d = din("biasf", [P, 11], f32)  # biasg(8) | e2db(2) | tconv(1)
    w1_d = din("w1T", [P, 8 * RS * 2 * P], None)  # fp8e4 tiles (ct, r, kh)

    # out: [p=(j4,q32), (g4, n2, blk16)] f32 partial channel-max
    out_d = nc.dram_tensor("part_out", [RS, B * 2 * 16], f32,
                           kind="ExternalOutput").ap()


    with tile.TileContext(nc) as tc:
        with (
            tc.tile_pool(name="const", bufs=1) as cpool,
            tc.tile_pool(name="xg", bufs=1) as xgpool,
            tc.tile_pool(name="hist", bufs=1) as hpool,
            tc.tile_pool(name="gs", bufs=4) as gspool,
            tc.tile_pool(name="cell", bufs=1) as cellpool,
            tc.tile_pool(name="tmp", bufs=6) as tmppool,
            tc.tile_pool(name="w1c", bufs=7) as w1pool,
            tc.tile_pool(name="f1", bufs=1) as f1pool,
            tc.tile_pool(name="rcv", bufs=1) as rpool,
            tc.tile_pool(name="feat", bufs=11) as fpool,
            tc.tile_pool(name="vout", bufs=1) as vpool,
            tc.tile_pool(name="mx", bufs=4) as mxpool,
        ):
            # ---- PE warm-up first: junk matmuls on uninitialized SBUF ----
            # (content irrelevant; just drive the HAM busy-window so the PE
            # is at 2.4GHz when stage A starts)
            warm = cpool.tile([P, P], bf16, tag="warmmm")
            nc.vector.memset(warm[:], 0.25)
            with tc.tile_pool(name="wpsum", bufs=2, space="PSUM") as wpsum:
                for i in range(46):
                    wp = wpsum.tile([P, P], f32, tag="warm")
                    nc.tensor.matmul(wp[:], lhsT=warm[:], rhs=warm[:],
                                     start=True, stop=True)

            pack = cpool.tile([P, sum(PK)], bf16, tag="cpack")
            nc.sync.dma_start(out=pack[:], in_=pack_d)
            biasf = cpool.tile([P, 11], f32, tag="biasf")
            nc.sync.dma_start(out=biasf[:], in_=biasf_d)

            off = np.cumsum([0] + PK)
            embT = pack[:, off[0]:off[1]]
            wih = pack[:, off[1]:off[2]]
            whh = pack[:, off[2]:off[3]]
            e2dT = pack[:, off[3]:off[4]]
            eye = pack[:, off[4]:off[5]]
            b1 = pack[0:RS, off[5]:off[6]]
            delta = pack[0:RS, off[6]:off[7]]
            tpat = pack[0:RS, off[7]:off[8]]      # [32, 2*128] rank-1 t rows
            ones_row = pack[0:RS, off[8]:off[9]]
            biasg = biasf[:, 0:8]
            e2db = biasf[:, 8:10]
            tconv = biasf[:, 10:11]

            # ---- feature DMA (16.8MB fp8; streams during LSTM) -----------
            fbs = {}
            for b in range(B):
                fbs[b] = fpool.tile([P, 8 * HW2], mybir.dt.float8e4,
                                    tag="feat", name=f"fb{b}")
                nc.sync.dma_start(
                    out=fbs[b][:].rearrange("p (kc hw) -> p kc hw", kc=8),
                    in_=feat_d[b])

            # ---- Stage A: xg = w_ih @ x_t for all steps (+ gate bias) ----
            xg_s = xgpool.tile([P, 8 * S * B], bf16)
            NCH = 320
            with tc.tile_pool(name="xpsum", bufs=2, space="PSUM") as xpsum:
                for m in range(8):
                    for n in range(2):
                        ps = xpsum.tile([P, NCH], f32, tag="xg")
                        for ke in range(2):
                            nc.tensor.matmul(
                                ps[:],
                                lhsT=wih[:, (ke * 8 + m) * P:(ke * 8 + m + 1) * P],
                                rhs=embT[:, ke * S * B + n * NCH: ke * S * B + (n + 1) * NCH],
                                start=(ke == 0), stop=(ke == 1),
                            )
                        nc.scalar.activation(
                            out=xg_s[:, m * S * B + n * NCH: m * S * B + (n + 1) * NCH],
                            in_=ps[:], func=AFT.Identity, bias=biasg[:, m:m + 1],
                        )

            # ---- Stage B: LSTM recurrence, 2 interleaved half-batches ----
            # Group g = samples 8g..8g+7.  While group 0's elementwise chain
            # runs on ACT/DVE, the PE does group 1's matmuls, halving the
            # per-step serial-latency cost.
            # Per group: gates psum gp [128, (m8, b8)]; work tile W [128,
            # (tg 16 | c 16)] so one 32-col DVE mult computes t1|t2 at once.
            GB = B // 2  # 8 samples per group
            hist = hpool.tile([P, S * 2 * B], bf16)   # (t, kh, b) b global
            Wk = {g: cellpool.tile([P, 4 * GB], f32, tag=f"W{g}",
                                   name=f"Wk{g}")
                  for g in range(2)}
            xg_r = xg_s[:].rearrange("p (m t b) -> p m t b", m=8, t=S)
            hist_r = hist[:].rearrange("p (t kh b) -> p t kh b", t=S, kh=2)
            S_g = [max(slots[g * GB:(g + 1) * GB]) + 1 for g in range(2)]
            lstm_psum = tc.tile_pool(name="gpsum", bufs=4, space="PSUM")
            gpsum = lstm_psum.__enter__()
            for t in range(max(S_g)):
                for g in range(2):
                    if t >= S_g[g]:
                        continue
                    bsl = slice(g * GB, (g + 1) * GB)
                    gp = gpsum.tile([P, 8 * GB], f32, tag="gates", name=f"gp{g}")
                    nc.tensor.matmul(gp[:], lhsT=eye[:],
                                     rhs=xg_r[:, :, t, bsl],
                                     start=True, stop=(t == 0))
                    if t > 0:
                        for m in range(8):
                            for kh in range(2):
                                nc.tensor.matmul(
                                    gp[:, m * GB:(m + 1) * GB],
                                    lhsT=whh[:, (kh * 8 + m) * P:
                                             (kh * 8 + m + 1) * P],
                                    rhs=hist_r[:, t - 1, kh, bsl],
                                    start=False, stop=(m == 7 and kh == 1),
                                    skip_group_check=True,
                                )
                    # cols (m, b8): i=0:16, f=16:32, g=32:48, o=48:64
                    gs = gspool.tile([P, 8 * GB], f32, tag=f"gs{g}")
                    nc.scalar.activation(out=gs[:], in_=gp[:],
                                         func=AFT.Sigmoid)
                    W = Wk[g]  # cols: tg 0:16 | c 16:32 (16 = kh2*b8)
                    nc.vector.tensor_scalar(W[:, 0:2 * GB], gs[:, 4 * GB:6 * GB],
                                            2.0, -1.0,
                                            AluOpType.mult, AluOpType.add)
                    U = tmppool.tile([P, 4 * GB], f32, tag=f"U{g}")
                    nc.vector.tensor_tensor(U[:, 0:4 * GB], gs[:, 0:4 * GB],
                                            W[:], AluOpType.mult)
                    if t == 0:
                        nc.vector.tensor_copy(W[:, 2 * GB:4 * GB],
                                              U[:, 0:2 * GB])
                    else:
                        nc.vector.tensor_tensor(W[:, 2 * GB:4 * GB],
                                                U[:, 0:2 * GB],
                                                U[:, 2 * GB:4 * GB],
                                                AluOpType.add)
                    th = tmppool.tile([P, 2 * GB], bf16, tag=f"th{g}")
                    nc.scalar.activation(out=th[:], in_=W[:, 2 * GB:4 * GB],
                                         func=AFT.Tanh)
                    nc.vector.tensor_tensor(
                        hist_r[:, t, :, bsl],
                        gs[:, 6 * GB:8 * GB].rearrange(
                            "p (kh b) -> p kh b", kh=2),
                        th[:].rearrange("p (kh b) -> p kh b", kh=2),
                        AluOpType.mult)


            # ---- capture final h per sample (compile-time slots) ---------
            h_fin = cellpool.tile([P, 2 * B], bf16, tag="hfin")  # (kh, b)
            hf_r = h_fin[:].rearrange("p (k b) -> p b k", k=2)
            for b in range(B):
                src = hist[:, slots[b] * 2 * B:(slots[b] + 1) * 2 * B]
                nc.gpsimd.tensor_copy(
                    hf_r[:, b], src.rearrange("p (k b) -> p b k", k=2)[:, b])

            # ---- e2d projection: instrT = tanh(e2d_w @ h + b) ------------
            instrT = cellpool.tile([P, 2 * B], bf16, tag="instrT")  # (kh, b)
            for m in range(2):
                pe2 = gpsum.tile([P, B], f32, tag="e2d")
                for kh in range(2):
                    nc.tensor.matmul(
                        pe2[:],
                        lhsT=e2dT[:, (kh * 2 + m) * P:(kh * 2 + m + 1) * P],
                        rhs=h_fin[:, kh * B:(kh + 1) * B],
                        start=(kh == 0), stop=(kh == 1),
                    )
                nc.scalar.activation(out=instrT[:, m * B:(m + 1) * B],
                                     in_=pe2[:], func=AFT.Tanh,
                                     bias=e2db[:, m:m + 1])
            lstm_psum.__exit__(None, None, None)

            # ---- lin1 (r-slice): f1[c, (ct,b,r)] = Lrelu(W @ instr + b1) -
            f1_sb = f1pool.tile([P, 8 * B * RS], mybir.dt.float8e4)
            CW = RS * 2 * P
            lin1_psum = tc.tile_pool(name="lpsum", bufs=4, space="PSUM")
            lpsum = lin1_psum.__enter__()
            for ct in range(8):
                wch = w1pool.tile([P, CW], mybir.dt.float8e4, tag="w1c")
                nc.sync.dma_start(out=wch[:], in_=w1_d[:, ct * CW:(ct + 1) * CW])
                pb = lpsum.tile([P, RS * B], f32, tag="lin1")
                nc.tensor.matmul(pb[:], lhsT=b1[:, ct * P:(ct + 1) * P],
                                 rhs=delta[:], start=True, stop=False,
                                 skip_group_check=True)
                for r in range(RS):
                    for kh in range(2):
                        nc.tensor.matmul(
                            pb[:, r * B:(r + 1) * B],
                            lhsT=wch[:, (r * 2 + kh) * P:(r * 2 + kh + 1) * P],
                            rhs=instrT[:, kh * B:(kh + 1) * B],
                            start=False, stop=(r == RS - 1 and kh == 1),
                            skip_group_check=True,
                        )
                out_ap = (f1_sb[:, ct * B * RS:(ct + 1) * B * RS]
                          .rearrange("p (b r) -> p r b", b=B))
                nc.scalar.activation(out=out_ap, in_=pb[:], func=AFT.Lrelu,
                                     alpha=0.01)
            lin1_psum.__exit__(None, None, None)
            # f1_sb col layout: ct*512 + b*32 + r

            # ---- conv + fused BN-shift + channel max (r-sharded) ---------
            # out[32r, hw] per sample; 4 samples packed in the 4 PE
            # column-quadrants via tile_position.  rhs = fp8 feature.
            vout = vpool.tile([RS, B * 2 * 16], f32)  # [q32, (b, n, blk)]
            conv_psum = tc.tile_pool(name="cpsum", bufs=4, space="PSUM")
            cpsum = conv_psum.__enter__()
            NH = HW2 // 2  # 512
            f1r = f1_sb[:].rearrange("p (kc b r) -> p kc b r", kc=8, b=B)
            for b in range(B):
                fb_r = fbs[b][:].rearrange("p (kc hw) -> p kc hw", kc=8)
                for n in range(2):
                    pc = cpsum.tile([RS, NH], f32, tag="conv")
                    for kp in range(4):
                        nc.tensor.matmul(
                            pc[:],
                            lhsT=f1r[:, 2 * kp:2 * kp + 2, b],
                            rhs=fb_r[:, 2 * kp:2 * kp + 2,
                                     n * NH:(n + 1) * NH],
                            start=(kp == 0), stop=(kp == 3),
                            skip_group_check=True,
                            perf_mode=mybir.MatmulPerfMode.DoubleRow,
                        )
                    cp = mxpool.tile([RS, NH], f32, tag="convcp")
                    nc.scalar.activation(out=cp[:], in_=pc[:],
                                         func=AFT.Identity,
                                         bias=tconv[0:RS])
                    nc.vector.tensor_reduce(
                        out=vout[0:RS, (b * 2 + n) * 16:(b * 2 + n + 1) * 16],
                        in_=cp[:].rearrange("p (blk q) -> p blk q", q=32),
                        axis=AX.X, op=AluOpType.max, apply_transpose=True)
            conv_psum.__exit__(None, None, None)

            nc.sync.dma_start(out=out_d, in_=vout[:])

    _split_excess_waits(nc)
    return nc


# ---------------------------------------------------------------------------
def _prep_inputs(feature, instruction_idx, instruction_length, emb_table,
                 w_ih, w_hh, b_ih, b_hh, e2d_w, e2d_b,
                 lin1_w, lin1_b, bn_gamma, bn_beta, bn_mean, bn_var):
    """Host-side layout/dtype prep. Returns (in_maps, slots, T0)."""
    f32 = np.float32

    def to_bf(x):
        return np.ascontiguousarray(x.astype(BF16))

    feature = np.asarray(feature, f32)
    emb_table = np.asarray(emb_table, f32)
    idx = np.asarray(instruction_idx)
    lengths = np.asarray(instruction_length).astype(np.int64)
    slots = [int(max(l, 1) - 1) for l in lengths]

    # feature [b, c_in(p), kc, hw] in fp8e4, replicated to all cores
    featr = feature.reshape(B, 8, P, HW2).transpose(0, 2, 1, 3)  # [B,P,8,HW2]
    feat8 = np.ascontiguousarray(featr.astype(ml_dtypes.float8_e4m3))

    # embeds transposed: [p, (ke, t*b)]
    emb = emb_table[idx]                       # [B, S, E]
    embT = emb.transpose(2, 1, 0).reshape(2, P, S * B)
    embT = to_bf(embT.transpose(1, 0, 2).reshape(P, 2 * S * B))

    def wtiles(w, kt, mt):
        wt = np.asarray(w, f32).T
        a = wt.reshape(kt, P, mt, P).transpose(1, 0, 2, 3)
        return to_bf(a.reshape(P, kt * mt * P))

    gsc = np.ones((4 * HID, 1), f32)
    gsc[2 * HID:3 * HID] = 2.0
    wihT = wtiles(np.asarray(w_ih, f32) * gsc, 2, 8)
    whhT = wtiles(np.asarray(w_hh, f32) * gsc, 2, 8)
    e2dT = wtiles(e2d_w, 2, 2)

    bg = ((np.asarray(b_ih, f32) + np.asarray(b_hh, f32)) * gsc[:, 0]) \
        .reshape(8, P).T.copy()
    e2db = np.asarray(e2d_b, f32).reshape(2, P).T.copy()

    s = np.asarray(bn_gamma, f32) / np.sqrt(np.asarray(bn_var, f32) + BN_EPS)
    tsh = np.asarray(bn_beta, f32) - np.asarray(bn_mean, f32) * s
    T0 = float(tsh.max())

    w1s = np.asarray(lin1_w, f32).reshape(R, C, HID) * s[:, None, None]
    b1s = np.asarray(lin1_b, f32).reshape(R, C) * s[:, None]

    delta = np.repeat(np.eye(RS, dtype=f32), B, axis=1)  # [32, 512]
    eye = np.eye(P, dtype=f32)
    ones32 = np.ones((RS, P), f32)

    def pad128(a):
        out = np.zeros((P, a.shape[1]), f32)
        out[:a.shape[0]] = a
        return out

    biasf = np.concatenate([bg, e2db], axis=1).astype(f32)

    ones_row = np.zeros((RS, HW2 // 2), f32)
    ones_row[0] = 1.0

    in_maps = []
    for k in range(N_CORES):
        rsl = slice(k * RS, (k + 1) * RS)
        wsl = w1s[rsl]                          # [32, 1024, 256] (r, c, h)
        ws = wsl.transpose(2, 1, 0)             # [h, c, r]
        a = (ws.reshape(2, P, 8, P, RS)         # [kh, p, ct, col, r]
             .transpose(1, 2, 4, 0, 3)          # [p, ct, r, kh, col]
             .reshape(P, 8 * RS * 2 * P))
        b1c = b1s[rsl].reshape(RS, 8, P).reshape(RS, 8 * P)  # (r, (ct, c))
        # conv BN-shift injection: out[p,:] += t[p%32] via rank-1 matmul
        tpat = np.zeros((RS, P), f32)
        tpat[0] = np.tile(tsh[rsl], 4)
        cpack = np.concatenate(
            [embT.astype(f32), wihT.astype(f32), whhT.astype(f32),
             e2dT.astype(f32), eye, pad128(b1c), pad128(delta),
             pad128(tpat), pad128(ones_row)], axis=1)
        tcol = np.zeros((P, 1), f32)
        tcol[:RS, 0] = tsh[rsl]
        biasf_k = np.ascontiguousarray(np.concatenate([biasf, tcol], axis=1))
        in_maps.append(dict(
            feat=feat8, cpack=to_bf(cpack), biasf=biasf_k,
            w1T=np.ascontiguousarray(a.astype(ml_dtypes.float8_e4m3))))
    return in_maps, slots, T0


_cache = {}


def _run(inputs, trace=False):
    (in_maps, slots, T0) = _prep_inputs(
        inputs["feature"], inputs["instruction_idx"],
        inputs["instruction_length"], inputs["emb_table"],
        inputs["w_ih"], inputs["w_hh"], inputs["b_ih"], inputs["b_hh"],
        inputs["e2d_w"], inputs["e2d_b"], inputs["lin1_w"], inputs["lin1_b"],
        inputs["bn_gamma"], inputs["bn_beta"], inputs["bn_mean"],
        inputs["bn_var"])

    key = tuple(slots)
    if key not in _cache:
        _cache[key] = _build_program(slots)
    nc = _cache[key]

    kw = {}
    if trace:
        kw = dict(trace=True, trace_cores=list(range(N_CORES)))
    res = run_bass_kernel_spmd(nc, in_maps, list(range(N_CORES)), **kw)
    parts = np.stack([np.asarray(res.results[i]["part_out"], np.float32)
                      for i in range(N_CORES)])
    v = parts.reshape(N_CORES, 32, B, 2, 16)      # [core, q, b, n, blk]
    v = v.transpose(0, 2, 3, 4, 1)                # [core, b, n, blk, q]
    single = v.reshape(N_CORES, B, HW2).max(axis=0)
    single = np.maximum(single, T0)
    out = np.clip(single, 0.0, 1.0).reshape(B, 32, 32).astype(np.float32)
    return out, res


def kernel(**inputs) -> np.ndarray:
    out, _ = _run(inputs, trace=False)
    return out


def kernel_traced(**inputs):
    out, res = _run(inputs, trace=True)
    return out, res


# revision 7
# speedup vs baseline: 1.2071x; 1.1845x over previous
"""Trainium2 Bass kernel for nn_Map_79748952752358 (dense_cnn), v2.

v3: r-sharded conv (as the original baseline) but with the replicated
feature tensor in fp8e4 (16.8MB instead of 33.5MB per core) and the lin1
weight slice in fp8e4 (8.4MB, fully prefetched).  No collectives: the
ncfw collective path measured 15-130us of nondeterministic wake/boot
latency, worse than simply halving the feature traffic with fp8.  LSTM
runs as 2 interleaved half-batches.

Key math folds (exact, done on host):
  - BN scale s=gamma/sqrt(var+eps) > 0 folded into lin1 weights/bias.
  - relu(x)+t maxed over r == max(max_r(x+t), max_r(t)); +t injected into
    the conv PSUM via a rank-1 matmul, the floor max_r(t) applied on host.
  - channel-max over 256 r is permutation invariant, so the AllToAll slot
    order (slot s = r-rows of source core s) needs no per-core fixup.
"""

import os
import numpy as np
import ml_dtypes

import concourse.bass as bass
import concourse.mybir as mybir
from concourse import tile
from concourse.tile import ScopedClock
from concourse.alu_op_type import AluOpType
from concourse.bass_utils import run_bass_kernel_spmd

BF16 = ml_dtypes.bfloat16

B, S, V, E, HID = 16, 40, 1004, 256, 256
C, R, HW2 = 1024, 256, 1024
BN_EPS = 1e-5
N_CORES = 8
RS = R // N_CORES    # 32 r-rows per core (lin1 shard)
BS = B // N_CORES    # 2 samples per core (conv shard)
P = 128
CH = RS * C * BS     # a2a block: 65536 bf16 elems = 128KB

AFT = mybir.ActivationFunctionType
AX = mybir.AxisListType


# ---------------------------------------------------------------------------
# Tile tail-drain patch: this walrus build accepts fewer sem waits per
# TPB_CTRL instruction than Tile's exit drain accumulates; split them into
# single-wait SP nops.
_drain_patched = False


def _patch_tile_drain():
    global _drain_patched
    if _drain_patched:
        return
    _drain_patched = True

    def _patched(self, tick_clock, wait_clock):
        nc = self.nc
        probe = nc.sync.nop(nofuse=True, hint="drain_wait_split")
        wait_clock.add_sem_waits(
            probe.ins, ScopedClock({None: tick_clock.global_clock})
        )
        si = probe.ins.sync_info
        waits = list(si.on_wait or []) if si is not None else []
        if len(waits) > 1:
            si.on_wait = waits[:1]
            for w in waits[1:]:
                n = nc.sync.nop(nofuse=True, hint="drain_wait_split")
                nsi = n.ins.sync_info
                if nsi is None:
                    import bass_rust

                    n.ins.sync_info = bass_rust.SyncInfo(on_wait=[w], on_update=[])
                else:
                    nsi.on_wait = [w]
        nc.sync.drain()
        nc.all_engine_barrier()
        assert self.sems is not None
        popped = nc._tile_sem_poison_stack.pop()
        assert popped is self._sem_poison
        nc.clear_and_free_semaphores(list(self.sems.allocated().values()))
        nc.all_engine_barrier()

    tile.TileContext._drain_and_barrier = _patched


_ws_counter = [0]


def _split_excess_waits(nc, limit=1):
    """Walrus on this image rejects instructions with more than ~2 sem waits.
    Move excess waits onto same-engine EventSemaphore carriers inserted just
    before the offending instruction."""
    import bass_rust

    for fn in nc.m.functions:
        for bb in fn.blocks:
            out = []
            for inst in bb.instructions:
                si = inst.sync_info
                waits = list(si.on_wait or []) if si is not None else []
                if len(waits) > limit:
                    for w in waits[:-limit]:
                        _ws_counter[0] += 1
                        carrier = mybir.InstEventSemaphore(
                            name=f"I-waitsplit-{_ws_counter[0]}",
                            opcode="EventSemaphore",
                            engine=inst.engine,
                            sync_info=bass_rust.SyncInfo(
                                on_wait=[w], on_update=[]),
                        )
                        out.append(carrier)
                    si.on_wait = waits[-limit:]
                out.append(inst)
            bb.instructions = out


# ---------------------------------------------------------------------------
def _build_program(slots):
    """Build the SPMD Bass program. `slots[b]` = length[b]-1 (compile-time)."""
    _patch_tile_drain()
    nc = bass.Bass("TRN2", target_bir_lowering=False, debug=False,
                   num_devices=N_CORES)
    dt = mybir.dt
    f32, bf16 = dt.float32, dt.bfloat16

    fp8 = dt.float8e4

    def din(name, shape, d=bf16):
        if d is None:
            d = fp8
        return nc.dram_tensor(name, shape, d, kind="ExternalInput").ap()

    # feature, all 16 samples, (b, c_in, c-tile, hw) fp8e4
    feat_d = din("feat", [B, P, 8, HW2], None)
    # packed bf16 constant block: embT | wihT | whhT | e2dT | eye |
    # b1 | delta | tpat | ones_row  (32-partition blocks zero-padded)
    PK = [2 * S * B, 16 * P, 16 * P, 4 * P, P, 8 * P, RS * B, P, HW2 // 2]
    pack_d = din("cpack", [P, sum(PK)])
    biasf_d = din("biasf", [P, 11], f32)  # biasg(8) | e2db(2) | tconv(1)
    w1_d = din("w1T", [P, 8 * RS * 2 * P], None)  # fp8e4 tiles (ct, r, kh)

    # out: [p=(j4,q32), (g4, n2, blk16)] f32 partial channel-max
    out_d = nc.dram_tensor("part_out", [RS, B * 2 * 16], f32,
                           kind="ExternalOutput").ap()


    with tile.TileContext(nc) as tc:
        with (
            tc.tile_pool(name="const", bufs=1) as cpool,
            tc.tile_pool(name="xg", bufs=1) as xgpool,
            tc.tile_pool(name="hist", bufs=1) as hpool,
            tc.tile_pool(name="gs", bufs=2) as gspool,
            tc.tile_pool(name="cell", bufs=1) as cellpool,
            tc.tile_pool(name="tmp", bufs=4) as tmppool,
            tc.tile_pool(name="w1c", bufs=7) as w1pool,
            tc.tile_pool(name="f1", bufs=1) as f1pool,
            tc.tile_pool(name="rcv", bufs=1) as rpool,
            tc.tile_pool(name="feat", bufs=11) as fpool,
            tc.tile_pool(name="vout", bufs=1) as vpool,
            tc.tile_pool(name="mx", bufs=4) as mxpool,
        ):
            # ---- PE warm-up first: junk matmuls on uninitialized SBUF ----
            # (content irrelevant; just drive the HAM busy-window so the PE
            # is at 2.4GHz when stage A starts)
            warm = cpool.tile([P, P], bf16, tag="warmmm")
            nc.vector.memset(warm[:], 0.25)
            with tc.tile_pool(name="wpsum", bufs=2, space="PSUM") as wpsum:
                for i in range(46):
                    wp = wpsum.tile([P, P], f32, tag="warm")
                    nc.tensor.matmul(wp[:], lhsT=warm[:], rhs=warm[:],
                                     start=True, stop=True)

            pack = cpool.tile([P, sum(PK)], bf16, tag="cpack")
            nc.sync.dma_start(out=pack[:], in_=pack_d)
            biasf = cpool.tile([P, 11], f32, tag="biasf")
            nc.sync.dma_start(out=biasf[:], in_=biasf_d)

            off = np.cumsum([0] + PK)
            embT = pack[:, off[0]:off[1]]
            wih = pack[:, off[1]:off[2]]
            whh = pack[:, off[2]:off[3]]
            e2dT = pack[:, off[3]:off[4]]
            eye = pack[:, off[4]:off[5]]
            b1 = pack[0:RS, off[5]:off[6]]
            delta = pack[0:RS, off[6]:off[7]]
            tpat = pack[0:RS, off[7]:off[8]]      # [32, 2*128] rank-1 t rows
            ones_row = pack[0:RS, off[8]:off[9]]
            biasg = biasf[:, 0:8]
            e2db = biasf[:, 8:10]
            tconv = biasf[:, 10:11]

            # ---- feature DMA (16.8MB fp8; streams during LSTM) -----------
            fbs = {}
            for b in range(B):
                fbs[b] = fpool.tile([P, 8 * HW2], mybir.dt.float8e4,
                                    tag="feat", name=f"fb{b}")
                nc.sync.dma_start(
                    out=fbs[b][:].rearrange("p (kc hw) -> p kc hw", kc=8),
                    in_=feat_d[b])

            # ---- Stage A: xg = w_ih @ x_t for all steps (+ gate bias) ----
            xg_s = xgpool.tile([P, 8 * S * B], bf16)
            NCH = 320
            with tc.tile_pool(name="xpsum", bufs=2, space="PSUM") as xpsum:
                for m in range(8):
                    for n in range(2):
                        ps = xpsum.tile([P, NCH], f32, tag="xg")
                        for ke in range(2):
                            nc.tensor.matmul(
                                ps[:],
                                lhsT=wih[:, (ke * 8 + m) * P:(ke * 8 + m + 1) * P],
                                rhs=embT[:, ke * S * B + n * NCH: ke * S * B + (n + 1) * NCH],
                                start=(ke == 0), stop=(ke == 1),
                            )
                        nc.scalar.activation(
                            out=xg_s[:, m * S * B + n * NCH: m * S * B + (n + 1) * NCH],
                            in_=ps[:], func=AFT.Identity, bias=biasg[:, m:m + 1],
                        )

            # ---- Stage B: LSTM recurrence, 2 interleaved half-batches ----
            # Group g = samples 8g..8g+7.  While group 0's elementwise chain
            # runs on ACT/DVE, the PE does group 1's matmuls, halving the
            # per-step serial-latency cost.
            # Per group: gates psum gp [128, (m8, b8)]; work tile W [128,
            # (tg 16 | c 16)] so one 32-col DVE mult computes t1|t2 at once.
            GB = B // 2  # 8 samples per group
            hist = hpool.tile([P, S * 2 * B], bf16)   # (t, kh, b) b global
            Wk = {g: cellpool.tile([P, 4 * GB], f32, tag=f"W{g}",
                                   name=f"Wk{g}")
                  for g in range(2)}
            xg_r = xg_s[:].rearrange("p (m t b) -> p m t b", m=8, t=S)
            hist_r = hist[:].rearrange("p (t kh b) -> p t kh b", t=S, kh=2)
            S_g = [max(slots[g * GB:(g + 1) * GB]) + 1 for g in range(2)]
            lstm_psum = tc.tile_pool(name="gpsum", bufs=4, space="PSUM")
            gpsum = lstm_psum.__enter__()
            for t in range(max(S_g)):
                for g in range(2):
                    if t >= S_g[g]:
                        continue
                    bsl = slice(g * GB, (g + 1) * GB)
                    gp = gpsum.tile([P, 8 * GB], f32, tag="gates", name=f"gp{g}")
                    nc.tensor.matmul(gp[:], lhsT=eye[:],
                                     rhs=xg_r[:, :, t, bsl],
                                     start=True, stop=(t == 0))
                    if t > 0:
                        for m in range(8):
                            for kh in range(2):
                                nc.tensor.matmul(
                                    gp[:, m * GB:(m + 1) * GB],
                                    lhsT=whh[:, (kh * 8 + m) * P:
                                             (kh * 8 + m + 1) * P],
                                    rhs=hist_r[:, t - 1, kh, bsl],
                                    start=False, stop=(m == 7 and kh == 1),
                                    skip_group_check=True,
                                )
                    # cols (m, b8): i=0:16, f=16:32, g=32:48, o=48:64
                    gs = gspool.tile([P, 8 * GB], f32, tag=f"gs{g}")
                    nc.scalar.activation(out=gs[:], in_=gp[:],
                                         func=AFT.Sigmoid)
                    W = Wk[g]  # cols: tg 0:16 | c 16:32 (16 = kh2*b8)
                    nc.vector.tensor_scalar(W[:, 0:2 * GB], gs[:, 4 * GB:6 * GB],
                                            2.0, -1.0,
                                            AluOpType.mult, AluOpType.add)
                    U = tmppool.tile([P, 4 * GB], f32, tag=f"U{g}")
                    nc.vector.tensor_tensor(U[:, 0:4 * GB], gs[:, 0:4 * GB],
                                            W[:], AluOpType.mult)
                    if t == 0:
                        nc.vector.tensor_copy(W[:, 2 * GB:4 * GB],
                                              U[:, 0:2 * GB])
                    else:
                        nc.vector.tensor_tensor(W[:, 2 * GB:4 * GB],
                                                U[:, 0:2 * GB],
                                                U[:, 2 * GB:4 * GB],
                                                AluOpType.add)
                    th = tmppool.tile([P, 2 * GB], bf16, tag=f"th{g}")
                    nc.scalar.activation(out=th[:], in_=W[:, 2 * GB:4 * GB],
                                         func=AFT.Tanh)
                    nc.vector.tensor_tensor(
                        hist_r[:, t, :, bsl],
                        gs[:, 6 * GB:8 * GB].rearrange(
                            "p (kh b) -> p kh b", kh=2),
                        th[:].rearrange("p (kh b) -> p kh b", kh=2),
                        AluOpType.mult)


            # ---- capture final h per sample (compile-time slots) ---------
            h_fin = cellpool.tile([P, 2 * B], bf16, tag="hfin")  # (kh, b)
            hf_r = h_fin[:].rearrange("p (k b) -> p b k", k=2)
            for b in range(B):
                src = hist[:, slots[b] * 2 * B:(slots[b] + 1) * 2 * B]
                nc.gpsimd.tensor_copy(
                    hf_r[:, b], src.rearrange("p (k b) -> p b k", k=2)[:, b])

            # ---- e2d projection: instrT = tanh(e2d_w @ h + b) ------------
            instrT = cellpool.tile([P, 2 * B], bf16, tag="instrT")  # (kh, b)
            for m in range(2):
                pe2 = gpsum.tile([P, B], f32, tag="e2d")
                for kh in range(2):
                    nc.tensor.matmul(
                        pe2[:],
                        lhsT=e2dT[:, (kh * 2 + m) * P:(kh * 2 + m + 1) * P],
                        rhs=h_fin[:, kh * B:(kh + 1) * B],
                        start=(kh == 0), stop=(kh == 1),
                    )
                nc.scalar.activation(out=instrT[:, m * B:(m + 1) * B],
                                     in_=pe2[:], func=AFT.Tanh,
                                     bias=e2db[:, m:m + 1])
            lstm_psum.__exit__(None, None, None)

            # ---- lin1 (r-slice): f1[c, (ct,b,r)] = Lrelu(W @ instr + b1) -
            f1_sb = f1pool.tile([P, 8 * B * RS], mybir.dt.float8e4)
            CW = RS * 2 * P
            lin1_psum = tc.tile_pool(name="lpsum", bufs=4, space="PSUM")
            lpsum = lin1_psum.__enter__()
            for ct in range(8):
                wch = w1pool.tile([P, CW], mybir.dt.float8e4, tag="w1c")
                nc.sync.dma_start(out=wch[:], in_=w1_d[:, ct * CW:(ct + 1) * CW])
                pb = lpsum.tile([P, RS * B], f32, tag="lin1")
                nc.tensor.matmul(pb[:], lhsT=b1[:, ct * P:(ct + 1) * P],
                                 rhs=delta[:], start=True, stop=False,
                                 skip_group_check=True)
                for r in range(RS):
                    for kh in range(2):
                        nc.tensor.matmul(
                            pb[:, r * B:(r + 1) * B],
                            lhsT=wch[:, (r * 2 + kh) * P:(r * 2 + kh + 1) * P],
                            rhs=instrT[:, kh * B:(kh + 1) * B],
                            start=False, stop=(r == RS - 1 and kh == 1),
                            skip_group_check=True,
                        )
                out_ap = (f1_sb[:, ct * B * RS:(ct + 1) * B * RS]
                          .rearrange("p (b r) -> p r b", b=B))
                nc.scalar.activation(out=out_ap, in_=pb[:], func=AFT.Lrelu,
                                     alpha=0.01)
            lin1_psum.__exit__(None, None, None)
            # f1_sb col layout: ct*512 + b*32 + r

            # ---- conv + fused BN-shift + channel max (r-sharded) ---------
            # out[32r, hw] per sample; 4 samples packed in the 4 PE
            # column-quadrants via tile_position.  rhs = fp8 feature.
            vout = vpool.tile([RS, B * 2 * 16], f32)  # [q32, (b, n, blk)]
            conv_psum = tc.tile_pool(name="cpsum", bufs=4, space="PSUM")
            cpsum = conv_psum.__enter__()
            NH = HW2 // 2  # 512
            f1r = f1_sb[:].rearrange("p (kc b r) -> p kc b r", kc=8, b=B)
            for b in range(B):
                fb_r = fbs[b][:].rearrange("p (kc hw) -> p kc hw", kc=8)
                for n in range(2):
                    pc = cpsum.tile([RS, NH], f32, tag="conv")
                    for kp in range(4):
                        nc.tensor.matmul(
                            pc[:],
                            lhsT=f1r[:, 2 * kp:2 * kp + 2, b],
                            rhs=fb_r[:, 2 * kp:2 * kp + 2,
                                     n * NH:(n + 1) * NH],
                            start=(kp == 0), stop=(kp == 3),
                            skip_group_check=True,
                            perf_mode=mybir.MatmulPerfMode.DoubleRow,
                        )
                    cp = mxpool.tile([RS, NH], f32, tag="convcp")
                    nc.scalar.activation(out=cp[:], in_=pc[:],
                                         func=AFT.Identity,
                                         bias=tconv[0:RS])
                    nc.vector.tensor_reduce(
                        out=vout[0:RS, (b * 2 + n) * 16:(b * 2 + n + 1) * 16],
                        in_=cp[:].rearrange("p (blk q) -> p blk q", q=32),
                        axis=AX.X, op=AluOpType.max, apply_transpose=True)
            conv_psum.__exit__(None, None, None)

            nc.sync.dma_start(out=out_d, in_=vout[:])

    _split_excess_waits(nc)
    return nc


# ---------------------------------------------------------------------------
def _prep_inputs(feature, instruction_idx, instruction_length, emb_table,
                 w_ih, w_hh, b_ih, b_hh, e2d_w, e2d_b,
                 lin1_w, lin1_b, bn_gamma, bn_beta, bn_mean, bn_var):
    """Host-side layout/dtype prep. Returns (in_maps, slots, T0)."""
    f32 = np.float32

    def to_bf(x):
        return np.ascontiguousarray(x.astype(BF16))

    feature = np.asarray(feature, f32)
    emb_table = np.asarray(emb_table, f32)
    idx = np.asarray(instruction_idx)
    lengths = np.asarray(instruction_length).astype(np.int64)
    slots = [int(max(l, 1) - 1) for l in lengths]

    # feature [b, c_in(p), kc, hw] in fp8e4, replicated to all cores
    featr = feature.reshape(B, 8, P, HW2).transpose(0, 2, 1, 3)  # [B,P,8,HW2]
    feat8 = np.ascontiguousarray(featr.astype(ml_dtypes.float8_e4m3))

    # embeds transposed: [p, (ke, t*b)]
    emb = emb_table[idx]                       # [B, S, E]
    embT = emb.transpose(2, 1, 0).reshape(2, P, S * B)
    embT = to_bf(embT.transpose(1, 0, 2).reshape(P, 2 * S * B))

    def wtiles(w, kt, mt):
        wt = np.asarray(w, f32).T
        a = wt.reshape(kt, P, mt, P).transpose(1, 0, 2, 3)
        return to_bf(a.reshape(P, kt * mt * P))

    gsc = np.ones((4 * HID, 1), f32)
    gsc[2 * HID:3 * HID] = 2.0
    wihT = wtiles(np.asarray(w_ih, f32) * gsc, 2, 8)
    whhT = wtiles(np.asarray(w_hh, f32) * gsc, 2, 8)
    e2dT = wtiles(e2d_w, 2, 2)

    bg = ((np.asarray(b_ih, f32) + np.asarray(b_hh, f32)) * gsc[:, 0]) \
        .reshape(8, P).T.copy()
    e2db = np.asarray(e2d_b, f32).reshape(2, P).T.copy()

    s = np.asarray(bn_gamma, f32) / np.sqrt(np.asarray(bn_var, f32) + BN_EPS)
    tsh = np.asarray(bn_beta, f32) - np.asarray(bn_mean, f32) * s
    T0 = float(tsh.max())

    w1s = np.asarray(lin1_w, f32).reshape(R, C, HID) * s[:, None, None]
    b1s = np.asarray(lin1_b, f32).reshape(R, C) * s[:, None]

    delta = np.repeat(np.eye(RS, dtype=f32), B, axis=1)  # [32, 512]
    eye = np.eye(P, dtype=f32)
    ones32 = np.ones((RS, P), f32)

    def pad128(a):
        out = np.zeros((P, a.shape[1]), f32)
        out[:a.shape[0]] = a
        return out

    biasf = np.concatenate([bg, e2db], axis=1).astype(f32)

    ones_row = np.zeros((RS, HW2 // 2), f32)
    ones_row[0] = 1.0

    in_maps = []
    for k in range(N_CORES):
        rsl = slice(k * RS, (k + 1) * RS)
        wsl = w1s[rsl]                          # [32, 1024, 256] (r, c, h)
        ws = wsl.transpose(2, 1, 0)             # [h, c, r]
        a = (ws.reshape(2, P, 8, P, RS)         # [kh, p, ct, col, r]
             .transpose(1, 2, 4, 0, 3)          # [p, ct, r, kh, col]
             .reshape(P, 8 * RS * 2 * P))
        b1c = b1s[rsl].reshape(RS, 8, P).reshape(RS, 8 * P)  # (r, (ct, c))
        # conv BN-shift injection: out[p,:] += t[p%32] via rank-1 matmul
        tpat = np.zeros((RS, P), f32)
        tpat[0] = np.tile(tsh[rsl], 4)
        cpack = np.concatenate(
            [embT.astype(f32), wihT.astype(f32), whhT.astype(f32),
             e2dT.astype(f32), eye, pad128(b1c), pad128(delta),
             pad128(tpat), pad128(ones_row)], axis=1)
        tcol = np.zeros((P, 1), f32)
        tcol[:RS, 0] = tsh[rsl]
        biasf_k = np.ascontiguousarray(np.concatenate([biasf, tcol], axis=1))
        in_maps.append(dict(
            feat=feat8, cpack=to_bf(cpack), biasf=biasf_k,
            w1T=np.ascontiguousarray(a.astype(ml_dtypes.float8_e4m3))))
    return in_maps, slots, T0


_cache = {}


def _run(inputs, trace=False):
    (in_maps, slots, T0) = _prep_inputs(
        inputs["feature"], inputs["instruction_idx"],
        inputs["instruction_length"], inputs["emb_table"],
        inputs["w_ih"], inputs["w_hh"], inputs["b_ih"], inputs["b_hh"],
        inputs["e2d_w"], inputs["e2d_b"], inputs["lin1_w"], inputs["lin1_b"],
        inputs["bn_gamma"], inputs["bn_beta"], inputs["bn_mean"],
        inputs["bn_var"])

    key = tuple(slots)
    if key not in _cache:
        _cache[key] = _build_program(slots)
    nc = _cache[key]

    kw = {}
    if trace:
        kw = dict(trace=True, trace_cores=list(range(N_CORES)))
    res = run_bass_kernel_spmd(nc, in_maps, list(range(N_CORES)), **kw)
    parts = np.stack([np.asarray(res.results[i]["part_out"], np.float32)
                      for i in range(N_CORES)])
    v = parts.reshape(N_CORES, 32, B, 2, 16)      # [core, q, b, n, blk]
    v = v.transpose(0, 2, 3, 4, 1)                # [core, b, n, blk, q]
    single = v.reshape(N_CORES, B, HW2).max(axis=0)
    single = np.maximum(single, T0)
    out = np.clip(single, 0.0, 1.0).reshape(B, 32, 32).astype(np.float32)
    return out, res


def kernel(**inputs) -> np.ndarray:
    out, _ = _run(inputs, trace=False)
    return out


def kernel_traced(**inputs):
    out, res = _run(inputs, trace=True)
    return out, res


# revision 9
# speedup vs baseline: 1.2157x; 1.0071x over previous
"""Trainium2 Bass kernel for nn_Map_79748952752358 (dense_cnn), v2.

v3: r-sharded conv (as the original baseline) but with the replicated
feature tensor in fp8e4 (16.8MB instead of 33.5MB per core) and the lin1
weight slice in fp8e4 (8.4MB, fully prefetched).  No collectives: the
ncfw collective path measured 15-130us of nondeterministic wake/boot
latency, worse than simply halving the feature traffic with fp8.  LSTM
runs as 2 interleaved half-batches.

Key math folds (exact, done on host):
  - BN scale s=gamma/sqrt(var+eps) > 0 folded into lin1 weights/bias.
  - relu(x)+t maxed over r == max(max_r(x+t), max_r(t)); +t injected into
    the conv PSUM via a rank-1 matmul, the floor max_r(t) applied on host.
  - channel-max over 256 r is permutation invariant, so the AllToAll slot
    order (slot s = r-rows of source core s) needs no per-core fixup.
"""

import os
import numpy as np
import ml_dtypes

import concourse.bass as bass
import concourse.mybir as mybir
from concourse import tile
from concourse.tile import ScopedClock
from concourse.alu_op_type import AluOpType
from concourse.bass_utils import run_bass_kernel_spmd

BF16 = ml_dtypes.bfloat16

B, S, V, E, HID = 16, 40, 1004, 256, 256
C, R, HW2 = 1024, 256, 1024
BN_EPS = 1e-5
N_CORES = 8
RS = R // N_CORES    # 32 r-rows per core (lin1 shard)
BS = B // N_CORES    # 2 samples per core (conv shard)
P = 128
CH = RS * C * BS     # a2a block: 65536 bf16 elems = 128KB

AFT = mybir.ActivationFunctionType
AX = mybir.AxisListType


# ---------------------------------------------------------------------------
# Tile tail-drain patch: this walrus build accepts fewer sem waits per
# TPB_CTRL instruction than Tile's exit drain accumulates; split them into
# single-wait SP nops.
_drain_patched = False


def _patch_tile_drain():
    global _drain_patched
    if _drain_patched:
        return
    _drain_patched = True

    def _patched(self, tick_clock, wait_clock):
        nc = self.nc
        probe = nc.sync.nop(nofuse=True, hint="drain_wait_split")
        wait_clock.add_sem_waits(
            probe.ins, ScopedClock({None: tick_clock.global_clock})
        )
        si = probe.ins.sync_info
        waits = list(si.on_wait or []) if si is not None else []
        if len(waits) > 1:
            si.on_wait = waits[:1]
            for w in waits[1:]:
                n = nc.sync.nop(nofuse=True, hint="drain_wait_split")
                nsi = n.ins.sync_info
                if nsi is None:
                    import bass_rust

                    n.ins.sync_info = bass_rust.SyncInfo(on_wait=[w], on_update=[])
                else:
                    nsi.on_wait = [w]
        nc.sync.drain()
        nc.all_engine_barrier()
        assert self.sems is not None
        popped = nc._tile_sem_poison_stack.pop()
        assert popped is self._sem_poison
        nc.clear_and_free_semaphores(list(self.sems.allocated().values()))
        nc.all_engine_barrier()

    tile.TileContext._drain_and_barrier = _patched


_ws_counter = [0]


def _split_excess_waits(nc, limit=1):
    """Walrus on this image rejects instructions with more than ~2 sem waits.
    Move excess waits onto same-engine EventSemaphore carriers inserted just
    before the offending instruction."""
    import bass_rust

    for fn in nc.m.functions:
        for bb in fn.blocks:
            out = []
            for inst in bb.instructions:
                si = inst.sync_info
                waits = list(si.on_wait or []) if si is not None else []
                if len(waits) > limit:
                    for w in waits[:-limit]:
                        _ws_counter[0] += 1
                        carrier = mybir.InstEventSemaphore(
                            name=f"I-waitsplit-{_ws_counter[0]}",
                            opcode="EventSemaphore",
                            engine=inst.engine,
                            sync_info=bass_rust.SyncInfo(
                                on_wait=[w], on_update=[]),
                        )
                        out.append(carrier)
                    si.on_wait = waits[-limit:]
                out.append(inst)
            bb.instructions = out


# ---------------------------------------------------------------------------
def _build_program(slots):
    """Build the SPMD Bass program. `slots[b]` = length[b]-1 (compile-time)."""
    _patch_tile_drain()
    nc = bass.Bass("TRN2", target_bir_lowering=False, debug=False,
                   num_devices=N_CORES)
    dt = mybir.dt
    f32, bf16 = dt.float32, dt.bfloat16

    fp8 = dt.float8e4

    def din(name, shape, d=bf16):
        if d is None:
            d = fp8
        return nc.dram_tensor(name, shape, d, kind="ExternalInput").ap()

    # feature, all 16 samples, (b, c_in, c-tile, hw) fp8e4
    feat_d = din("feat", [B, P, 8, HW2], None)
    # packed bf16 constant block: embT | wihT | whhT | e2dT | eye |
    # b1 | delta | tpat | ones_row  (32-partition blocks zero-padded)
    PK = [2 * S * B, 16 * P, 16 * P, 4 * P, P, 8 * P, RS * B, P, HW2 // 2]
    pack_d = din("cpack", [P, sum(PK)])
    biasf_d = din("biasf", [P, 11], f32)  # biasg(8) | e2db(2) | tconv(1)
    w1_d = din("w1T", [P, 8 * RS * 2 * P], None)  # fp8e4 tiles (ct, r, kh)

    # out: [p=(j4,q32), (g4, n2, blk16)] f32 partial channel-max
    out_d = nc.dram_tensor("part_out", [RS, B * 2 * 16], f32,
                           kind="ExternalOutput").ap()


    with tile.TileContext(nc) as tc:
        with (
            tc.tile_pool(name="const", bufs=1) as cpool,
            tc.tile_pool(name="xg", bufs=1) as xgpool,
            tc.tile_pool(name="hist", bufs=1) as hpool,
            tc.tile_pool(name="gs", bufs=2) as gspool,
            tc.tile_pool(name="cell", bufs=1) as cellpool,
            tc.tile_pool(name="tmp", bufs=4) as tmppool,
            tc.tile_pool(name="w1c", bufs=7) as w1pool,
            tc.tile_pool(name="f1", bufs=1) as f1pool,
            tc.tile_pool(name="rcv", bufs=1) as rpool,
            tc.tile_pool(name="feat", bufs=11) as fpool,
            tc.tile_pool(name="vout", bufs=1) as vpool,
            tc.tile_pool(name="mx", bufs=4) as mxpool,
        ):
            # ---- PE warm-up first: junk matmuls on uninitialized SBUF ----
            # (content irrelevant; just drive the HAM busy-window so the PE
            # is at 2.4GHz when stage A starts)
            warm = cpool.tile([P, P], bf16, tag="warmmm")
            nc.vector.memset(warm[:], 0.25)
            with tc.tile_pool(name="wpsum", bufs=2, space="PSUM") as wpsum:
                for i in range(46):
                    wp = wpsum.tile([P, P], f32, tag="warm")
                    nc.tensor.matmul(wp[:], lhsT=warm[:], rhs=warm[:],
                                     start=True, stop=True)

            pack = cpool.tile([P, sum(PK)], bf16, tag="cpack")
            nc.sync.dma_start(out=pack[:], in_=pack_d)
            biasf = cpool.tile([P, 11], f32, tag="biasf")
            nc.sync.dma_start(out=biasf[:], in_=biasf_d)

            off = np.cumsum([0] + PK)
            embT = pack[:, off[0]:off[1]]
            wih = pack[:, off[1]:off[2]]
            whh = pack[:, off[2]:off[3]]
            e2dT = pack[:, off[3]:off[4]]
            eye = pack[:, off[4]:off[5]]
            b1 = pack[0:RS, off[5]:off[6]]
            delta = pack[0:RS, off[6]:off[7]]
            tpat = pack[0:RS, off[7]:off[8]]      # [32, 2*128] rank-1 t rows
            ones_row = pack[0:RS, off[8]:off[9]]
            biasg = biasf[:, 0:8]
            e2db = biasf[:, 8:10]
            tconv = biasf[:, 10:11]

            # ---- feature DMA (16.8MB fp8; streams during LSTM) -----------
            fbs = {}
            for b in range(B):
                fbs[b] = fpool.tile([P, 8 * HW2], mybir.dt.float8e4,
                                    tag="feat", name=f"fb{b}")
                nc.sync.dma_start(
                    out=fbs[b][:].rearrange("p (kc hw) -> p kc hw", kc=8),
                    in_=feat_d[b])

            # ---- Stage A: xg = w_ih @ x_t for all steps (+ gate bias) ----
            xg_s = xgpool.tile([P, 8 * S * B], bf16)
            NCH = 320
            with tc.tile_pool(name="xpsum", bufs=2, space="PSUM") as xpsum:
                for m in range(8):
                    for n in range(2):
                        ps = xpsum.tile([P, NCH], f32, tag="xg")
                        for ke in range(2):
                            nc.tensor.matmul(
                                ps[:],
                                lhsT=wih[:, (ke * 8 + m) * P:(ke * 8 + m + 1) * P],
                                rhs=embT[:, ke * S * B + n * NCH: ke * S * B + (n + 1) * NCH],
                                start=(ke == 0), stop=(ke == 1),
                            )
                        nc.scalar.activation(
                            out=xg_s[:, m * S * B + n * NCH: m * S * B + (n + 1) * NCH],
                            in_=ps[:], func=AFT.Identity, bias=biasg[:, m:m + 1],
                        )

            # ---- Stage B: LSTM recurrence, 2 interleaved half-batches ----
            # Group g = samples 8g..8g+7.  While group 0's elementwise chain
            # runs on ACT/DVE, the PE does group 1's matmuls, halving the
            # per-step serial-latency cost.
            # Per group: gates psum gp [128, (m8, b8)]; work tile W [128,
            # (tg 16 | c 16)] so one 32-col DVE mult computes t1|t2 at once.
            GB = B // 2  # 8 samples per group
            hist = hpool.tile([P, S * 2 * B], bf16)   # (t, kh, b) b global
            Wk = {g: cellpool.tile([P, 4 * GB], f32, tag=f"W{g}",
                                   name=f"Wk{g}")
                  for g in range(2)}
            xg_r = xg_s[:].rearrange("p (m t b) -> p m t b", m=8, t=S)
            hist_r = hist[:].rearrange("p (t kh b) -> p t kh b", t=S, kh=2)
            S_g = [max(slots[g * GB:(g + 1) * GB]) + 1 for g in range(2)]
            lstm_psum = tc.tile_pool(name="gpsum", bufs=4, space="PSUM")
            gpsum = lstm_psum.__enter__()
            for t in range(max(S_g)):
                for g in range(2):
                    if t >= S_g[g]:
                        continue
                    bsl = slice(g * GB, (g + 1) * GB)
                    gp = gpsum.tile([P, 8 * GB], f32, tag="gates", name=f"gp{g}")
                    nc.tensor.matmul(gp[:], lhsT=eye[:],
                                     rhs=xg_r[:, :, t, bsl],
                                     start=True, stop=(t == 0))
                    if t > 0:
                        for m in range(8):
                            for kh in range(2):
                                nc.tensor.matmul(
                                    gp[:, m * GB:(m + 1) * GB],
                                    lhsT=whh[:, (kh * 8 + m) * P:
                                             (kh * 8 + m + 1) * P],
                                    rhs=hist_r[:, t - 1, kh, bsl],
                                    start=False, stop=(m == 7 and kh == 1),
                                    skip_group_check=True,
                                )
                    # cols (m, b8): i=0:16, f=16:32, g=32:48, o=48:64
                    gs = gspool.tile([P, 8 * GB], f32, tag=f"gs{g}")
                    nc.scalar.activation(out=gs[:], in_=gp[:],
                                         func=AFT.Sigmoid)
                    W = Wk[g]  # cols: tg 0:16 | c 16:32 (16 = kh2*b8)
                    nc.vector.tensor_scalar(W[:, 0:2 * GB], gs[:, 4 * GB:6 * GB],
                                            2.0, -1.0,
                                            AluOpType.mult, AluOpType.add)
                    U = tmppool.tile([P, 4 * GB], f32, tag=f"U{g}")
                    nc.vector.tensor_tensor(U[:, 0:4 * GB], gs[:, 0:4 * GB],
                                            W[:], AluOpType.mult)
                    if t == 0:
                        nc.vector.tensor_copy(W[:, 2 * GB:4 * GB],
                                              U[:, 0:2 * GB])
                    else:
                        nc.vector.tensor_tensor(W[:, 2 * GB:4 * GB],
                                                U[:, 0:2 * GB],
                                                U[:, 2 * GB:4 * GB],
                                                AluOpType.add)
                    th = tmppool.tile([P, 2 * GB], bf16, tag=f"th{g}")
                    nc.scalar.activation(out=th[:], in_=W[:, 2 * GB:4 * GB],
                                         func=AFT.Tanh)
                    nc.vector.tensor_tensor(
                        hist_r[:, t, :, bsl],
                        gs[:, 6 * GB:8 * GB].rearrange(
                            "p (kh b) -> p kh b", kh=2),
                        th[:].rearrange("p (kh b) -> p kh b", kh=2),
                        AluOpType.mult)


            # ---- capture final h per sample (compile-time slots) ---------
            h_fin = cellpool.tile([P, 2 * B], bf16, tag="hfin")  # (kh, b)
            hf_r = h_fin[:].rearrange("p (k b) -> p b k", k=2)
            for b in range(B):
                src = hist[:, slots[b] * 2 * B:(slots[b] + 1) * 2 * B]
                nc.gpsimd.tensor_copy(
                    hf_r[:, b], src.rearrange("p (k b) -> p b k", k=2)[:, b])

            # ---- e2d projection: instrT = tanh(e2d_w @ h + b) ------------
            instrT = cellpool.tile([P, 2 * B], bf16, tag="instrT")  # (kh, b)
            for m in range(2):
                pe2 = gpsum.tile([P, B], f32, tag="e2d")
                for kh in range(2):
                    nc.tensor.matmul(
                        pe2[:],
                        lhsT=e2dT[:, (kh * 2 + m) * P:(kh * 2 + m + 1) * P],
                        rhs=h_fin[:, kh * B:(kh + 1) * B],
                        start=(kh == 0), stop=(kh == 1),
                    )
                nc.scalar.activation(out=instrT[:, m * B:(m + 1) * B],
                                     in_=pe2[:], func=AFT.Tanh,
                                     bias=e2db[:, m:m + 1])
            lstm_psum.__exit__(None, None, None)

            # ---- lin1 (r-slice): f1[c, (ct,b,r)] = Lrelu(W @ instr + b1) -
            f1_sb = f1pool.tile([P, 8 * B * RS], mybir.dt.float8e4)
            CW = RS * 2 * P
            lin1_psum = tc.tile_pool(name="lpsum", bufs=4, space="PSUM")
            lpsum = lin1_psum.__enter__()
            for ct in range(8):
                wch = w1pool.tile([P, CW], mybir.dt.float8e4, tag="w1c")
                nc.sync.dma_start(out=wch[:], in_=w1_d[:, ct * CW:(ct + 1) * CW])
                pb = lpsum.tile([P, RS * B], f32, tag="lin1")
                nc.tensor.matmul(pb[:], lhsT=b1[:, ct * P:(ct + 1) * P],
                                 rhs=delta[:], start=True, stop=False,
                                 skip_group_check=True)
                for r in range(RS):
                    for kh in range(2):
                        nc.tensor.matmul(
                            pb[:, r * B:(r + 1) * B],
                            lhsT=wch[:, (r * 2 + kh) * P:(r * 2 + kh + 1) * P],
                            rhs=instrT[:, kh * B:(kh + 1) * B],
                            start=False, stop=(r == RS - 1 and kh == 1),
                            skip_group_check=True,
                        )
                out_ap = (f1_sb[:, ct * B * RS:(ct + 1) * B * RS]
                          .rearrange("p (b r) -> p r b", b=B))
                nc.scalar.activation(out=out_ap, in_=pb[:], func=AFT.Lrelu,
                                     alpha=0.01)
            lin1_psum.__exit__(None, None, None)
            # f1_sb col layout: ct*512 + b*32 + r

            # ---- conv + fused BN-shift + channel max (r-sharded) ---------
            # out[32r, hw] per sample; 4 samples packed in the 4 PE
            # column-quadrants via tile_position.  rhs = fp8 feature.
            vout = vpool.tile([RS, B * 2 * 16], f32)  # [q32, (b, n, blk)]
            conv_psum = tc.tile_pool(name="cpsum", bufs=4, space="PSUM")
            cpsum = conv_psum.__enter__()
            NH = HW2 // 2  # 512
            f1r = f1_sb[:].rearrange("p (kc b r) -> p kc b r", kc=8, b=B)
            for b in range(B):
                fb_r = fbs[b][:].rearrange("p (kc hw) -> p kc hw", kc=8)
                for n in range(2):
                    pc = cpsum.tile([RS, NH], f32, tag="conv")
                    for kp in range(4):
                        nc.tensor.matmul(
                            pc[:],
                            lhsT=f1r[:, 2 * kp:2 * kp + 2, b],
                            rhs=fb_r[:, 2 * kp:2 * kp + 2,
                                     n * NH:(n + 1) * NH],
                            start=(kp == 0), stop=(kp == 3),
                            skip_group_check=True,
                            perf_mode=mybir.MatmulPerfMode.DoubleRow,
                        )
                    cp = mxpool.tile([RS, NH], f32, tag="convcp")
                    nc.scalar.activation(out=cp[:], in_=pc[:],
                                         func=AFT.Identity,
                                         bias=tconv[0:RS])
                    nc.vector.tensor_reduce(
                        out=vout[0:RS, (b * 2 + n) * 16:(b * 2 + n + 1) * 16],
                        in_=cp[:].rearrange("p (blk q) -> p blk q", q=32),
                        axis=AX.X, op=AluOpType.max, apply_transpose=True)
            conv_psum.__exit__(None, None, None)

            nc.sync.dma_start(out=out_d, in_=vout[:])

    _split_excess_waits(nc)
    return nc


# ---------------------------------------------------------------------------
def _prep_inputs(feature, instruction_idx, instruction_length, emb_table,
                 w_ih, w_hh, b_ih, b_hh, e2d_w, e2d_b,
                 lin1_w, lin1_b, bn_gamma, bn_beta, bn_mean, bn_var):
    """Host-side layout/dtype prep. Returns (in_maps, slots, T0)."""
    f32 = np.float32

    def to_bf(x):
        return np.ascontiguousarray(x.astype(BF16))

    feature = np.asarray(feature, f32)
    emb_table = np.asarray(emb_table, f32)
    idx = np.asarray(instruction_idx)
    lengths = np.asarray(instruction_length).astype(np.int64)
    slots = [int(max(l, 1) - 1) for l in lengths]

    # feature [b, c_in(p), kc, hw] in fp8e4, replicated to all cores
    featr = feature.reshape(B, 8, P, HW2).transpose(0, 2, 1, 3)  # [B,P,8,HW2]
    feat8 = np.ascontiguousarray(featr.astype(ml_dtypes.float8_e4m3))

    # embeds transposed: [p, (ke, t*b)]
    emb = emb_table[idx]                       # [B, S, E]
    embT = emb.transpose(2, 1, 0).reshape(2, P, S * B)
    embT = to_bf(embT.transpose(1, 0, 2).reshape(P, 2 * S * B))

    def wtiles(w, kt, mt):
        wt = np.asarray(w, f32).T
        a = wt.reshape(kt, P, mt, P).transpose(1, 0, 2, 3)
        return to_bf(a.reshape(P, kt * mt * P))

    gsc = np.ones((4 * HID, 1), f32)
    gsc[2 * HID:3 * HID] = 2.0
    wihT = wtiles(np.asarray(w_ih, f32) * gsc, 2, 8)
    whhT = wtiles(np.asarray(w_hh, f32) * gsc, 2, 8)
    e2dT = wtiles(e2d_w, 2, 2)

    bg = ((np.asarray(b_ih, f32) + np.asarray(b_hh, f32)) * gsc[:, 0]) \
        .reshape(8, P).T.copy()
    e2db = np.asarray(e2d_b, f32).reshape(2, P).T.copy()

    s = np.asarray(bn_gamma, f32) / np.sqrt(np.asarray(bn_var, f32) + BN_EPS)
    tsh = np.asarray(bn_beta, f32) - np.asarray(bn_mean, f32) * s
    T0 = float(tsh.max())

    w1s = np.asarray(lin1_w, f32).reshape(R, C, HID) * s[:, None, None]
    b1s = np.asarray(lin1_b, f32).reshape(R, C) * s[:, None]

    delta = np.repeat(np.eye(RS, dtype=f32), B, axis=1)  # [32, 512]
    eye = np.eye(P, dtype=f32)
    ones32 = np.ones((RS, P), f32)

    def pad128(a):
        out = np.zeros((P, a.shape[1]), f32)
        out[:a.shape[0]] = a
        return out

    biasf = np.concatenate([bg, e2db], axis=1).astype(f32)

    ones_row = np.zeros((RS, HW2 // 2), f32)
    ones_row[0] = 1.0

    in_maps = []
    for k in range(N_CORES):
        rsl = slice(k * RS, (k + 1) * RS)
        wsl = w1s[rsl]                          # [32, 1024, 256] (r, c, h)
        ws = wsl.transpose(2, 1, 0)             # [h, c, r]
        a = (ws.reshape(2, P, 8, P, RS)         # [kh, p, ct, col, r]
             .transpose(1, 2, 4, 0, 3)          # [p, ct, r, kh, col]
             .reshape(P, 8 * RS * 2 * P))
        b1c = b1s[rsl].reshape(RS, 8, P).reshape(RS, 8 * P)  # (r, (ct, c))
        # conv BN-shift injection: out[p,:] += t[p%32] via rank-1 matmul
        tpat = np.zeros((RS, P), f32)
        tpat[0] = np.tile(tsh[rsl], 4)
        cpack = np.concatenate(
            [embT.astype(f32), wihT.astype(f32), whhT.astype(f32),
             e2dT.astype(f32), eye, pad128(b1c), pad128(delta),
             pad128(tpat), pad128(ones_row)], axis=1)
        tcol = np.zeros((P, 1), f32)
        tcol[:RS, 0] = tsh[rsl]
        biasf_k = np.ascontiguousarray(np.concatenate([biasf, tcol], axis=1))
        in_maps.append(dict(
            feat=feat8, cpack=to_bf(cpack), biasf=biasf_k,
            w1T=np.ascontiguousarray(a.astype(ml_dtypes.float8_e4m3))))
    return in_maps, slots, T0


_cache = {}


def _run(inputs, trace=False):
    (in_maps, slots, T0) = _prep_inputs(
        inputs["feature"], inputs["instruction_idx"],
        inputs["instruction_length"], inputs["emb_table"],
        inputs["w_ih"], inputs["w_hh"], inputs["b_ih"], inputs["b_hh"],
        inputs["e2d_w"], inputs["e2d_b"], inputs["lin1_w"], inputs["lin1_b"],
        inputs["bn_gamma"], inputs["bn_beta"], inputs["bn_mean"],
        inputs["bn_var"])

    key = tuple(slots)
    if key not in _cache:
        _cache[key] = _build_program(slots)
    nc = _cache[key]

    kw = {}
    if trace:
        kw = dict(trace=True, trace_cores=list(range(N_CORES)))
    res = run_bass_kernel_spmd(nc, in_maps, list(range(N_CORES)), **kw)
    parts = np.stack([np.asarray(res.results[i]["part_out"], np.float32)
                      for i in range(N_CORES)])
    v = parts.reshape(N_CORES, 32, B, 2, 16)      # [core, q, b, n, blk]
    v = v.transpose(0, 2, 3, 4, 1)                # [core, b, n, blk, q]
    single = v.reshape(N_CORES, B, HW2).max(axis=0)
    single = np.maximum(single, T0)
    out = np.clip(single, 0.0, 1.0).reshape(B, 32, 32).astype(np.float32)
    return out, res


def kernel(**inputs) -> np.ndarray:
    out, _ = _run(inputs, trace=False)
    return out


def kernel_traced(**inputs):
    out, res = _run(inputs, trace=True)
    return out, res
